# revision 23
# baseline (speedup 1.0000x reference)
"""Trainium2 Bass kernel for nn_MixedGatedMLP (4-bit quantized gated MLP + LoRA).

Strategy: tensor-parallel over d_ff across 8 NeuronCores (F padded 11008->11264,
1408 rows/core). Host ships the 4-bit weights as unpacked u8 nibble planes plus
blockwise-scale planes; each core dequantizes on-device with a fused custom DVE
op chain (8 instructions per plane: acc = (X==2j)*c_{2j} + (X==2j+1)*c_{2j+1}
+ acc, exact in bf16), runs the three matmuls in bf16 on TensorE with LoRA
folded in as extra accumulation matmuls, applies silu-gating, and the partial
down-proj outputs are combined with a ReduceScatter. Core i returns final
tokens [512*i, 512*(i+1)); the host concatenates and casts to f32.

Dequant instructions for the next weight slice are interleaved between matmul
groups so TensorE stays busy (HAM-warm) throughout; PSUM banks are recycled
via ACT-engine copies so TensorE never waits on the (busier) vector engine.
"""

import sys

for _p in ("/opt/trn_rl_repo", "/root/.axon_site/_ro/trn_rl_repo"):
    if _p not in sys.path:
        sys.path.append(_p)

from contextlib import ExitStack

import numpy as np
import ml_dtypes

import concourse.bass as bass
import concourse.mybir as mybir
import concourse.tile as tile
from concourse import bacc
from concourse.bass_utils import run_bass_kernel_spmd

BF16 = ml_dtypes.bfloat16
NCORES = 8
ALU = mybir.AluOpType
AFT = mybir.ActivationFunctionType

# --------------- custom DVE ops: fused pair-LUT steps ---------------

import concourse.dve_ops as _dvo
from concourse.dve_spec import Spec, Src0, Src1, C0, C1, C2, One, eq, lower
from concourse.dve_uop import DveOpSpec


def _pair_ref(in0, in1, s0, s1, imm2, with_acc):
    x = np.asarray(in0).astype(np.float32)
    r = (x == float(imm2)).astype(np.float32) * np.asarray(s0, np.float32) \
        + (x == float(imm2) + 1.0).astype(np.float32) * np.asarray(s1, np.float32)
    if with_acc:
        r = r + np.asarray(in1).astype(np.float32)
    return r


def _register_dve_op(name, spec):
    for op in _dvo.OPS:
        if op.name == name:
            return op
    row = _dvo._CUSTOM_DVE_ROW_BASE + len(_dvo.OPS)
    assert row < 0x20
    shas = {}
    for ver in ("v3", "v4"):
        try:
            uops = lower(spec, ver=ver)
        except Exception:
            continue
        shas[ver] = DveOpSpec(
            name=name, opcode=row, uops=uops, rd1_en=_dvo.has_src1(spec)
        ).sha(ver)
    op = _dvo.DveOp(name, spec, subdim=False, uops_sha=shas)
    _dvo.OPS.append(op)
    _dvo._SUB_OPCODE_FOR_NAME[name] = row
    _dvo.CUSTOM_DVE_SPECS[name] = spec
    return op


_PAIR_BODY = eq(Src0, C2) * C0 + eq(Src0, C2 + One) * C1

DEQ_PAIR = _register_dve_op(
    "DEQ_PAIR_ANT",
    Spec(
        body=_PAIR_BODY,
        reference=lambda in0, in1, s0, s1, imm2: _pair_ref(
            in0, in1, s0, s1, imm2, False
        ),
    ),
)
DEQ_PAIR_ACC = _register_dve_op(
    "DEQ_PAIR_ACC_ANT",
    Spec(
        body=_PAIR_BODY + Src1,
        reference=lambda in0, in1, s0, s1, imm2: _pair_ref(
            in0, in1, s0, s1, imm2, True
        ),
    ),
)


class Cfg:
    def __init__(self, D=4096, T=4096, F=11008, R=16, block=64, ncores=8):
        self.D = D              # d_model
        self.T = T              # tokens
        self.F = F              # true d_ff
        self.R = R              # lora rank
        self.block = block      # absmax block size
        self.ncores = ncores
        self.FP = 11264         # padded d_ff (8*1408)
        self.FS = self.FP // ncores          # per-core f rows (1408 = 11*128)
        self.TS = T // ncores                # per-core output tokens
        self.DP = D // 128                   # 128-d chunks (32)
        self.TT = 256                        # phase-1 token tile
        self.NT = T // self.TT               # token tiles (16)
        self.NFG = self.FS // 128            # 128-f groups (11)
        # phase-1 f slices; widths are multiples of 128
        self.f_slices = [(0, 128), (128, 256), (384, 384), (768, 384),
                         (1152, 256)]
        # chain sub-tile width (per custom-DVE instruction): custom DVE ops
        # pay ~240ns fixed overhead each, so wider is better
        self.chain_fd = 1024
        # phase-2 d slices
        self.DDQ = 512
        self.n_q = D // self.DDQ
        self.use_rs = True                   # ReduceScatter (else A2A+adds)


def build_graph(cfg: Cfg):
    nc = bacc.Bacc(None, num_devices=cfg.ncores)
    dt = mybir.dt
    D, T, FS, R, TT = cfg.D, cfg.T, cfg.FS, cfg.R, cfg.TT

    # ---- external inputs (per-core) ----
    xT = nc.dram_tensor("xT", [D, T], dt.bfloat16, kind="ExternalInput")
    g_idx = nc.dram_tensor("g_idx", [D, FS], dt.uint8, kind="ExternalInput")
    u_idx = nc.dram_tensor("u_idx", [D, FS], dt.uint8, kind="ExternalInput")
    d_idx = nc.dram_tensor("d_idx", [FS, D], dt.uint8, kind="ExternalInput")
    g_s = nc.dram_tensor("g_s", [D, FS], dt.bfloat16, kind="ExternalInput")
    u_s = nc.dram_tensor("u_s", [D, FS], dt.bfloat16, kind="ExternalInput")
    d_s = nc.dram_tensor("d_s", [FS, D], dt.bfloat16, kind="ExternalInput")
    code_rep = nc.dram_tensor("code_rep", [128, 16], dt.float32, kind="ExternalInput")
    a_gu = nc.dram_tensor("a_gu", [D, 2 * R], dt.bfloat16, kind="ExternalInput")
    b_g = nc.dram_tensor("b_g", [R, FS], dt.bfloat16, kind="ExternalInput")
    b_u = nc.dram_tensor("b_u", [R, FS], dt.bfloat16, kind="ExternalInput")
    a_d = nc.dram_tensor("a_d", [FS, R], dt.bfloat16, kind="ExternalInput")
    b_d = nc.dram_tensor("b_d", [R, D], dt.bfloat16, kind="ExternalInput")

    y_q = [
        nc.dram_tensor(f"y_q{j}", [cfg.TS, cfg.DDQ], dt.bfloat16,
                       kind="ExternalOutput")
        for j in range(cfg.n_q)
    ]

    # ---- internal DRAM ----
    x3_dram = nc.dram_tensor("x3_dram", [FS, T], dt.bfloat16, kind="Internal")
    xag_dram = nc.dram_tensor("xag_dram", [R, T], dt.bfloat16, kind="Internal")
    xau_dram = nc.dram_tensor("xau_dram", [R, T], dt.bfloat16, kind="Internal")
    rs_in = [
        nc.dram_tensor(f"rs_in{i}", [T, cfg.DDQ], dt.bfloat16, kind="Internal")
        for i in range(2)
    ]
    rs_out = [
        nc.dram_tensor(f"rs_out{i}", [cfg.TS, cfg.DDQ], dt.bfloat16,
                       kind="Internal")
        for i in range(cfg.n_q)
    ]

    rg = [list(range(cfg.ncores))]

    with tile.TileContext(nc) as tc, ExitStack() as ctx:
        const_pool = ctx.enter_context(tc.tile_pool(name="const", bufs=1))
        code_sb = const_pool.tile([128, 16], dt.float32)
        nc.sync.dma_start(code_sb[:], code_rep[:])
        agu_sb = const_pool.tile([128, D // 128, 2 * R], dt.bfloat16)
        nc.sync.dma_start(agu_sb[:], a_gu.rearrange("(c p) r -> p c r", p=128))
        bg_sb = const_pool.tile([R, FS], dt.bfloat16)
        nc.sync.dma_start(bg_sb[:], b_g[:])
        bu_sb = const_pool.tile([R, FS], dt.bfloat16)
        nc.sync.dma_start(bu_sb[:], b_u[:])
        ad_sb = const_pool.tile([128, cfg.NFG, R], dt.bfloat16)
        nc.sync.dma_start(ad_sb[:], a_d.rearrange("(c p) r -> p c r", p=128))
        x3a_sb = const_pool.tile([R, T], dt.bfloat16)

        dq_pool = ctx.enter_context(tc.tile_pool(name="dq", bufs=2))
        # down-proj weights for q0/q1 (dequanted during phase-1 tail)
        wd0_pool = ctx.enter_context(tc.tile_pool(name="wd0",
                                                  bufs=2 * cfg.NFG))
        wd_ready = {}

        def chain_lut(X_f, S_f, w_f, fw):
            """LUT-dequant flat [128, fw] APs: u8 nibbles -> bf16 weights."""
            pool = dq_pool
            step = min(cfg.chain_fd, fw)
            for c0 in range(0, fw, step):
                cw = min(step, fw - c0)
                cs = slice(c0, c0 + cw)
                acc = pool.tile([128, step], dt.bfloat16, tag="dqa", name="acc")
                w = acc[:, 0:cw]
                nc.vector._custom_dve(
                    DEQ_PAIR, out=w, in0=X_f[:, cs],
                    s0=code_sb[:, 0:1], s1=code_sb[:, 1:2], imm2=0.0)
                for j in range(1, 8):
                    acc2 = pool.tile([128, step], dt.bfloat16, tag="dqa",
                                     name="acc2")
                    w2 = acc2[:, 0:cw]
                    nc.vector._custom_dve(
                        DEQ_PAIR_ACC, out=w2, in0=X_f[:, cs], in1=w,
                        s0=code_sb[:, 2 * j:2 * j + 1],
                        s1=code_sb[:, 2 * j + 1:2 * j + 2], imm2=float(2 * j))
                    w = w2
                nc.vector.tensor_tensor(w_f[:, cs], w, S_f[:, cs], ALU.mult)

        def dequant_block(wpool, idx_ap3, s_ap3, c2, fw, tag, nm,
                          wshape=None, wtag=None, xs_bufs=3):
            """Load c2 128-row chunks x fw cols, dequant as one wide plane.
            Returns the [128, c2, fw] weight tile."""
            pw = wshape and [128, wshape[0] * wshape[1]]
            X = dq_pool.tile([128, c2 * fw], dt.uint8, tag=f"X{tag}",
                             name=f"X{nm}", padded_shape=pw, bufs=xs_bufs)
            S = dq_pool.tile([128, c2 * fw], dt.bfloat16, tag=f"S{tag}",
                             name=f"S{nm}", padded_shape=pw, bufs=xs_bufs)
            dma_eng = nc.gpsimd if tag == "d" else None
            (dma_eng or nc.sync).dma_start(
                X.rearrange("p (c f) -> p c f", c=c2), idx_ap3)
            (dma_eng or nc.scalar).dma_start(
                S.rearrange("p (c f) -> p c f", c=c2), s_ap3)
            wt = wpool.tile([128, c2 * fw], dt.bfloat16,
                            tag=f"w{wtag or tag}", name=f"w{nm}",
                            padded_shape=pw)
            chain_lut(X[:], S[:], wt[:], c2 * fw)
            return wt.rearrange("p (c f) -> p c f", c=c2)

        def dq_phase2_gen(q, pool):
            """Dequant the down-proj chunks for quarter q (one plane/yield)."""
            dd0 = cfg.DDQ * q
            wd = {}
            for g in range(cfg.NFG):
                rows = slice(128 * g, 128 * (g + 1))
                cols = slice(dd0, dd0 + cfg.DDQ)
                wt = dequant_block(
                    pool,
                    d_idx[rows, cols].rearrange("(c p) f -> p c f", p=128),
                    d_s[rows, cols].rearrange("(c p) f -> p c f", p=128),
                    1, cfg.DDQ, "d", f"wd{q}_{g}", xs_bufs=6)
                wd[g] = wt[:, 0, :]
                yield 1
            wd_ready[q] = wd

        # =============== phase 1: gate/up matmuls -> x3 ===============
        with (
            tc.tile_pool(name="w", bufs=2 * (cfg.DP // 4) + 1) as w_pool,
            tc.tile_pool(name="xt", bufs=2) as xt_pool,
            tc.tile_pool(name="p1", bufs=2) as p1_pool,
            tc.tile_pool(name="ps1", bufs=2, space="PSUM") as psum1,
            tc.tile_pool(name="psa", bufs=2, space="PSUM") as psuma,
        ):
            # dequant generator: one gate-or-up 4-chunk block per next()
            C2B = 4
            def dq_slice_gen(f0, fw, wg, wu):
                for c0 in range(0, cfg.DP, C2B):
                    rows = slice(128 * c0, 128 * (c0 + C2B))
                    cols = slice(f0, f0 + fw)
                    for (ti, tsrc, wdict, nm) in (
                        (g_idx, g_s, wg, "g"), (u_idx, u_s, wu, "u"),
                    ):
                        wt = dequant_block(
                            w_pool,
                            ti[rows, cols].rearrange("(c p) f -> p c f",
                                                     p=128),
                            tsrc[rows, cols].rearrange("(c p) f -> p c f",
                                                       p=128),
                            C2B, fw, "gu", f"w{nm}{c0}",
                            wshape=[C2B, 384], wtag=nm)
                        for j in range(C2B):
                            wdict[c0 + j] = wt[:, j, :]
                        yield 1

            slices = cfg.f_slices
            wbufs = [({}, {}) for _ in slices]
            # slice 0 dequant up-front (overlaps the lora prepass MMs)
            for _ in dq_slice_gen(slices[0][0], slices[0][1],
                                  wbufs[0][0], wbufs[0][1]):
                pass

            # interleavable dequant work: slices 1..3, then down-proj q0
            def pending_dq():
                for s in range(1, len(slices)):
                    yield from dq_slice_gen(slices[s][0], slices[s][1],
                                            wbufs[s][0], wbufs[s][1])
                yield from dq_phase2_gen(0, wd0_pool)
                yield from dq_phase2_gen(1, wd0_pool)

            dq_iter = pending_dq()
            upsl = 2 * cfg.DP // C2B          # units per slice (16)
            dq_units = (len(slices) - 1) * upsl + 2 * cfg.NFG
            mm_groups_total = sum(fw // 128 for _, fw in slices) * cfg.NT
            done = [0, 0]  # units, groups

            def advance_dq(want):
                want = min(dq_units, want)
                while done[0] < want:
                    if next(dq_iter, None) is None:
                        done[0] = dq_units
                        break
                    done[0] += 1

            for s, (f0, fw) in enumerate(slices):
                # slice s's weights must be fully emitted before its MMs
                advance_dq(s * upsl)
                wg, wu = wbufs[s]
                ng = fw // 128
                for t in range(cfg.NT):
                    tt = slice(TT * t, TT * (t + 1))
                    xts_all = xt_pool.tile([128, cfg.DP, TT], dt.bfloat16,
                                           tag="xt", name="xts_all")
                    nc.sync.dma_start(
                        xts_all[:],
                        xT[:, tt].rearrange("(c p) t -> p c t", p=128))
                    xts = [xts_all[:, ci, :] for ci in range(cfg.DP)]
                    if s == 0:
                        # lora x@A for this token tile (overlaps dequant)
                        for ri, dst in ((0, xag_dram), (1, xau_dram)):
                            pa = psuma.tile([R, TT], dt.float32, tag="pa")
                            for ci in range(cfg.DP):
                                nc.tensor.matmul(
                                    pa[:], agu_sb[:, ci, R * ri:R * (ri + 1)],
                                    xts[ci][:],
                                    start=(ci == 0), stop=(ci == cfg.DP - 1))
                            st = p1_pool.tile([R, TT], dt.bfloat16, tag="st")
                            nc.scalar.copy(st[:], pa[:])
                            nc.scalar.dma_start(dst[:, tt], st[:])
                    xag_t = p1_pool.tile([R, TT], dt.bfloat16, tag="xag_t")
                    nc.sync.dma_start(xag_t[:], xag_dram[:, tt])
                    xau_t = p1_pool.tile([R, TT], dt.bfloat16, tag="xau_t")
                    nc.sync.dma_start(xau_t[:], xau_dram[:, tt])
                    pa2 = psuma.tile([R, TT], dt.float32, tag="pa2")
                    x3t_t = p1_pool.tile([128, ng, TT], dt.bfloat16,
                                         tag="x3t", padded_shape=[128, 3, TT])
                    for g in range(ng):
                        fg = slice(128 * g, 128 * (g + 1))
                        fga = slice(f0 + 128 * g, f0 + 128 * (g + 1))
                        pg = psum1.tile([128, TT], dt.float32, tag="pg")
                        pu = psum1.tile([128, TT], dt.float32, tag="pu")
                        for ci in range(cfg.DP):
                            nc.tensor.matmul(pg[:], wg[ci][:, fg], xts[ci][:],
                                             start=(ci == 0), stop=False)
                        nc.tensor.matmul(pg[:], bg_sb[:, fga], xag_t[:],
                                         start=False, stop=True)
                        for ci in range(cfg.DP):
                            nc.tensor.matmul(pu[:], wu[ci][:, fg], xts[ci][:],
                                             start=(ci == 0), stop=False)
                        nc.tensor.matmul(pu[:], bu_sb[:, fga], xau_t[:],
                                         start=False, stop=True)
                        sg = p1_pool.tile([128, TT], dt.bfloat16, tag="sg")
                        nc.scalar.activation(sg[:], pg[:], AFT.Silu)
                        pu_sb = p1_pool.tile([128, TT], dt.bfloat16,
                                             tag="pu_sb")
                        nc.scalar.copy(pu_sb[:], pu[:])
                        nc.vector.tensor_tensor(x3t_t[:, g, :], sg[:],
                                                pu_sb[:], ALU.mult)
                        done[1] += 1
                        advance_dq((done[1] * 3) // 4)
                    nc.scalar.dma_start(
                        x3_dram[f0:f0 + fw, tt].rearrange(
                            "(g p) t -> p g t", p=128),
                        x3t_t[:])
                    # x3a partial: accumulate a_d over this slice's groups
                    for g in range(ng):
                        g_abs = (f0 // 128) + g
                        nc.tensor.matmul(pa2[:], ad_sb[:, g_abs, :],
                                         x3t_t[:, g, :],
                                         start=(g == 0), stop=(g == ng - 1))
                    # fold the x3a partial into the running accumulator
                    x3a_p = p1_pool.tile([R, TT], dt.bfloat16, tag="x3ap")
                    nc.scalar.copy(x3a_p[:], pa2[:])
                    if s == 0:
                        nc.scalar.copy(x3a_sb[:, tt], x3a_p[:])
                    else:
                        nc.vector.tensor_tensor(x3a_sb[:, tt], x3a_sb[:, tt],
                                                x3a_p[:], ALU.add)
            for _ in dq_iter:   # drain remaining dequant (down-proj q0)
                pass

        # =============== phase 2: down matmul + ReduceScatter ===============
        with (
            tc.tile_pool(name="wd", bufs=2 * cfg.NFG + 2) as wd_pool,
            tc.tile_pool(name="p2", bufs=2) as p2_pool,
            tc.tile_pool(name="yb2", bufs=2) as yb_pool,
            tc.tile_pool(name="lor", bufs=2) as lor_pool,
            tc.tile_pool(name="ps2", bufs=3, space="PSUM") as psum2,
        ):
            ph2_iter = None
            ph2_done = [0]
            pending_rs = [None]

            def emit_rs():
                if pending_rs[0] is not None:
                    qq = pending_rs[0]
                    nc.gpsimd.collective_compute(
                        "ReduceScatter", ALU.add, replica_groups=rg,
                        ins=[rs_in[qq % 2][:, :].opt()],
                        outs=[rs_out[qq][:, :].opt()],
                    )
                    pending_rs[0] = None

            for q in range(cfg.n_q):
                dd0 = cfg.DDQ * q
                wd = wd_ready[q]
                if q + 2 < cfg.n_q:
                    ph2_iter = dq_phase2_gen(q + 2, wd_pool)
                    ph2_done[0] = 0
                bdt = lor_pool.tile([R, cfg.DDQ], dt.bfloat16, tag="bdt")
                nc.sync.dma_start(bdt[:], b_d[:, dd0:dd0 + cfg.DDQ])
                n_tb = cfg.T // 512
                for tb in range(n_tb):
                    tsl = slice(512 * tb, 512 * (tb + 1))
                    x3b_all = p2_pool.tile([128, cfg.NFG, 512], dt.bfloat16,
                                           tag="x3l", name="x3b_all")
                    nc.sync.dma_start(
                        x3b_all[:],
                        x3_dram[:, tsl].rearrange("(g p) t -> p g t", p=128))
                    x3b = [x3b_all[:, g, :] for g in range(cfg.NFG)]
                    if tb == 1:
                        emit_rs()   # previous quarter's RS, after q's first
                                    # loads are already in the queues
                    for tk in range(4):
                        tg = 4 * tb + tk
                        tks = slice(128 * tk, 128 * (tk + 1))
                        tgs = slice(128 * tg, 128 * (tg + 1))
                        pd = psum2.tile([128, 512], dt.float32, tag="pd")
                        for g in range(cfg.NFG):
                            nc.tensor.matmul(
                                pd[:], x3b[g][:, tks], wd[g][:],
                                start=(g == 0), stop=False)
                        nc.tensor.matmul(
                            pd[:], x3a_sb[:, tgs], bdt[:],
                            start=False, stop=True)
                        yb = yb_pool.tile([128, cfg.DDQ], dt.bfloat16,
                                          tag="yb")
                        nc.scalar.copy(yb[:], pd[:])
                        nc.scalar.dma_start(rs_in[q % 2][tgs, :], yb[:])
                    # pace the next-next slice's dequant
                    if ph2_iter is not None:
                        want = min(cfg.NFG, ((tb + 1) * cfg.NFG) // 5)
                        while ph2_done[0] < want:
                            if next(ph2_iter, None) is None:
                                ph2_done[0] = cfg.NFG
                                break
                            ph2_done[0] += 1
                if ph2_iter is not None:
                    for _ in ph2_iter:
                        pass
                    ph2_iter = None
                pending_rs[0] = q
            emit_rs()
            for q in range(cfg.n_q):
                nc.sync.dma_start(y_q[q][:, :], rs_out[q][:, :])

    nc.compile()
    return nc


# ----------------- host side -----------------

_CACHE = {}


def _get_graph(cfg: Cfg):
    key = (cfg.D, cfg.T, cfg.F, cfg.ncores)
    if key not in _CACHE:
        _CACHE[key] = build_graph(cfg)
    return _CACHE[key]


def _unpack_idx(packed, rows, cols):
    """bnb layout: byte i of row-major [rows, cols] holds elements (2i, 2i+1)
    as (hi, lo) nibbles. Returns [rows, cols] u8 of 4-bit indices."""
    b = (packed.astype(np.int64) & 0xFF).astype(np.uint8).reshape(rows, cols // 2)
    out = np.empty((rows, cols), np.uint8)
    out[:, 0::2] = b >> 4
    out[:, 1::2] = b & 0xF
    return out


def _prep_inputs(cfg: Cfg, inputs):
    """Shard + lay out the full inputs for each core. Marshalling only."""
    D, T, F, FP, FS, R = cfg.D, cfg.T, cfg.F, cfg.FP, cfg.FS, cfg.R
    blk = cfg.block

    x = inputs["x"]
    xT = np.ascontiguousarray(np.asarray(x).T).astype(BF16)

    gi_full = _unpack_idx(np.asarray(inputs["w_gate_packed"]), F, D)
    ui_full = _unpack_idx(np.asarray(inputs["w_up_packed"]), F, D)
    di_full = _unpack_idx(np.asarray(inputs["w_down_packed"]), D, F)
    ga_full = np.asarray(inputs["w_gate_absmax"]).reshape(F, D // blk).astype(np.float32)
    ua_full = np.asarray(inputs["w_up_absmax"]).reshape(F, D // blk).astype(np.float32)
    da_full = np.asarray(inputs["w_down_absmax"]).reshape(D, F // blk).astype(np.float32)

    code_rep = np.broadcast_to(
        np.asarray(inputs["code"]).astype(np.float32)[None, :], (128, 16)
    ).copy()
    a_gu = np.concatenate(
        [inputs["w_gate_lora_a"], inputs["w_up_lora_a"]], axis=1
    ).astype(BF16)

    def pad_cols(m, w):
        return np.concatenate(
            [m, np.zeros((m.shape[0], w - m.shape[1]), m.dtype)], 1)

    b_g_full = pad_cols(np.asarray(inputs["w_gate_lora_b"], np.float32), FP)
    b_u_full = pad_cols(np.asarray(inputs["w_up_lora_b"], np.float32), FP)
    a_d_full = np.concatenate(
        [np.asarray(inputs["w_down_lora_a"], np.float32),
         np.zeros((FP - F, R), np.float32)], 0)
    b_d = np.asarray(inputs["w_down_lora_b"]).astype(BF16)

    in_maps = []
    for i in range(cfg.ncores):
        f0, f1 = FS * i, FS * (i + 1)
        nf = max(0, min(f1, F) - f0)        # valid f rows in this shard

        def row_shard(idx_full, am_full):
            """gate/up: -> idx [D, FS] u8, S [D, FS] bf16."""
            gi = np.zeros((FS, D), np.uint8)
            gs = np.zeros((FS, D), np.float32)
            if nf > 0:
                gi[:nf] = idx_full[f0:f0 + nf]
                gs[:nf] = np.repeat(am_full[f0:f0 + nf], blk, axis=1)
            return (np.ascontiguousarray(gi.T),
                    np.ascontiguousarray(gs.T.astype(BF16)))

        def col_shard(idx_full, am_full):
            """down: -> idx [FS, D] u8, S [FS, D] bf16."""
            di = np.zeros((D, FS), np.uint8)
            dsc = np.zeros((D, FS), np.float32)
            if nf > 0:
                di[:, :nf] = idx_full[:, f0:f0 + nf]
                nb = (nf + blk - 1) // blk
                amr = np.repeat(am_full[:, f0 // blk:f0 // blk + nb],
                                blk, axis=1)[:, :nf]
                dsc[:, :nf] = amr
            return (np.ascontiguousarray(di.T),
                    np.ascontiguousarray(dsc.T.astype(BF16)))

        g_i, g_sc = row_shard(gi_full, ga_full)
        u_i, u_sc = row_shard(ui_full, ua_full)
        d_i, d_sc = col_shard(di_full, da_full)

        in_maps.append({
            "xT": xT,
            "g_idx": g_i, "g_s": g_sc,
            "u_idx": u_i, "u_s": u_sc,
            "d_idx": d_i, "d_s": d_sc,
            "code_rep": code_rep,
            "a_gu": a_gu,
            "b_g": np.ascontiguousarray(b_g_full[:, f0:f1]).astype(BF16),
            "b_u": np.ascontiguousarray(b_u_full[:, f0:f1]).astype(BF16),
            "a_d": np.ascontiguousarray(a_d_full[f0:f1]).astype(BF16),
            "b_d": b_d,
        })
    return in_maps


def run(cfg: Cfg, inputs, trace=False, **kwargs):
    nc = _get_graph(cfg)
    in_maps = _prep_inputs(cfg, inputs)
    res = run_bass_kernel_spmd(
        nc, in_maps, core_ids=list(range(cfg.ncores)), trace=trace, **kwargs
    )
    y = np.concatenate(
        [np.concatenate([res.results[i][f"y_q{j}"] for j in range(cfg.n_q)],
                        axis=1)
         for i in range(cfg.ncores)], 0)
    return y, res


def kernel(**inputs) -> np.ndarray:
    cfg = Cfg()
    y, _ = run(cfg, inputs)
    return y.astype(np.float32)


# revision 24
# speedup vs baseline: 1.0244x; 1.0244x over previous
"""Trainium2 Bass kernel for nn_MixedGatedMLP (4-bit quantized gated MLP + LoRA).

Strategy: tensor-parallel over d_ff across 8 NeuronCores (F padded 11008->11264,
1408 rows/core). Host ships the 4-bit weights as unpacked u8 nibble planes plus
blockwise-scale planes; each core dequantizes on-device with a fused custom DVE
op chain (8 instructions per plane: acc = (X==2j)*c_{2j} + (X==2j+1)*c_{2j+1}
+ acc, exact in bf16), runs the three matmuls in bf16 on TensorE with LoRA
folded in as extra accumulation matmuls, applies silu-gating, and the partial
down-proj outputs are combined with a ReduceScatter. Core i returns final
tokens [512*i, 512*(i+1)); the host concatenates and casts to f32.

Dequant instructions for the next weight slice are interleaved between matmul
groups so TensorE stays busy (HAM-warm) throughout; PSUM banks are recycled
via ACT-engine copies so TensorE never waits on the (busier) vector engine.
"""

import sys

for _p in ("/opt/trn_rl_repo", "/root/.axon_site/_ro/trn_rl_repo"):
    if _p not in sys.path:
        sys.path.append(_p)

from contextlib import ExitStack

import numpy as np
import ml_dtypes

import concourse.bass as bass
import concourse.mybir as mybir
import concourse.tile as tile
from concourse import bacc
from concourse.bass_utils import run_bass_kernel_spmd

BF16 = ml_dtypes.bfloat16
NCORES = 8
ALU = mybir.AluOpType
AFT = mybir.ActivationFunctionType

# --------------- custom DVE ops: fused pair-LUT steps ---------------

import concourse.dve_ops as _dvo
from concourse.dve_spec import Spec, Src0, Src1, C0, C1, C2, One, eq, lower
from concourse.dve_uop import DveOpSpec


def _pair_ref(in0, in1, s0, s1, imm2, with_acc):
    x = np.asarray(in0).astype(np.float32)
    r = (x == float(imm2)).astype(np.float32) * np.asarray(s0, np.float32) \
        + (x == float(imm2) + 1.0).astype(np.float32) * np.asarray(s1, np.float32)
    if with_acc:
        r = r + np.asarray(in1).astype(np.float32)
    return r


def _register_dve_op(name, spec):
    for op in _dvo.OPS:
        if op.name == name:
            return op
    row = _dvo._CUSTOM_DVE_ROW_BASE + len(_dvo.OPS)
    assert row < 0x20
    shas = {}
    for ver in ("v3", "v4"):
        try:
            uops = lower(spec, ver=ver)
        except Exception:
            continue
        shas[ver] = DveOpSpec(
            name=name, opcode=row, uops=uops, rd1_en=_dvo.has_src1(spec)
        ).sha(ver)
    op = _dvo.DveOp(name, spec, subdim=False, uops_sha=shas)
    _dvo.OPS.append(op)
    _dvo._SUB_OPCODE_FOR_NAME[name] = row
    _dvo.CUSTOM_DVE_SPECS[name] = spec
    return op


_PAIR_BODY = eq(Src0, C2) * C0 + eq(Src0, C2 + One) * C1

DEQ_PAIR = _register_dve_op(
    "DEQ_PAIR_ANT",
    Spec(
        body=_PAIR_BODY,
        reference=lambda in0, in1, s0, s1, imm2: _pair_ref(
            in0, in1, s0, s1, imm2, False
        ),
    ),
)
DEQ_PAIR_ACC = _register_dve_op(
    "DEQ_PAIR_ACC_ANT",
    Spec(
        body=_PAIR_BODY + Src1,
        reference=lambda in0, in1, s0, s1, imm2: _pair_ref(
            in0, in1, s0, s1, imm2, True
        ),
    ),
)


class Cfg:
    def __init__(self, D=4096, T=4096, F=11008, R=16, block=64, ncores=8):
        self.D = D              # d_model
        self.T = T              # tokens
        self.F = F              # true d_ff
        self.R = R              # lora rank
        self.block = block      # absmax block size
        self.ncores = ncores
        self.FP = 11264         # padded d_ff (8*1408)
        self.FS = self.FP // ncores          # per-core f rows (1408 = 11*128)
        self.TS = T // ncores                # per-core output tokens
        self.DP = D // 128                   # 128-d chunks (32)
        self.TT = 256                        # phase-1 token tile
        self.NT = T // self.TT               # token tiles (16)
        self.NFG = self.FS // 128            # 128-f groups (11)
        # phase-1 f slices; widths are multiples of 128
        self.f_slices = [(0, 128), (128, 256), (384, 384), (768, 384),
                         (1152, 256)]
        # chain sub-tile width (per custom-DVE instruction): custom DVE ops
        # pay ~240ns fixed overhead each, so wider is better
        self.chain_fd = 1024
        # phase-2 d slices
        self.DDQ = 512
        self.n_q = D // self.DDQ
        self.use_rs = True                   # ReduceScatter (else A2A+adds)


def build_graph(cfg: Cfg):
    nc = bacc.Bacc(None, num_devices=cfg.ncores)
    dt = mybir.dt
    D, T, FS, R, TT = cfg.D, cfg.T, cfg.FS, cfg.R, cfg.TT

    # ---- external inputs (per-core) ----
    xT = nc.dram_tensor("xT", [D, T], dt.bfloat16, kind="ExternalInput")
    g_idx = nc.dram_tensor("g_idx", [D, FS], dt.uint8, kind="ExternalInput")
    u_idx = nc.dram_tensor("u_idx", [D, FS], dt.uint8, kind="ExternalInput")
    d_idx = nc.dram_tensor("d_idx", [FS, D], dt.uint8, kind="ExternalInput")
    g_s = nc.dram_tensor("g_s", [D, FS], dt.bfloat16, kind="ExternalInput")
    u_s = nc.dram_tensor("u_s", [D, FS], dt.bfloat16, kind="ExternalInput")
    d_s = nc.dram_tensor("d_s", [FS, D], dt.bfloat16, kind="ExternalInput")
    code_rep = nc.dram_tensor("code_rep", [128, 16], dt.float32, kind="ExternalInput")
    a_gu = nc.dram_tensor("a_gu", [D, 2 * R], dt.bfloat16, kind="ExternalInput")
    b_g = nc.dram_tensor("b_g", [R, FS], dt.bfloat16, kind="ExternalInput")
    b_u = nc.dram_tensor("b_u", [R, FS], dt.bfloat16, kind="ExternalInput")
    a_d = nc.dram_tensor("a_d", [FS, R], dt.bfloat16, kind="ExternalInput")
    b_d = nc.dram_tensor("b_d", [R, D], dt.bfloat16, kind="ExternalInput")

    y_q = [
        nc.dram_tensor(f"y_q{j}", [cfg.TS, cfg.DDQ], dt.bfloat16,
                       kind="ExternalOutput")
        for j in range(cfg.n_q)
    ]

    # ---- internal DRAM ----
    x3_dram = nc.dram_tensor("x3_dram", [FS, T], dt.bfloat16, kind="Internal")
    xag_dram = nc.dram_tensor("xag_dram", [R, T], dt.bfloat16, kind="Internal")
    xau_dram = nc.dram_tensor("xau_dram", [R, T], dt.bfloat16, kind="Internal")
    rs_in = [
        nc.dram_tensor(f"rs_in{i}", [T, cfg.DDQ], dt.bfloat16, kind="Internal")
        for i in range(cfg.n_q)
    ]
    rs_out = [
        nc.dram_tensor(f"rs_out{i}", [cfg.TS, cfg.DDQ], dt.bfloat16,
                       kind="Internal")
        for i in range(cfg.n_q)
    ]

    rg = [list(range(cfg.ncores))]

    with tile.TileContext(nc) as tc, ExitStack() as ctx:
        const_pool = ctx.enter_context(tc.tile_pool(name="const", bufs=1))
        code_sb = const_pool.tile([128, 16], dt.float32)
        nc.sync.dma_start(code_sb[:], code_rep[:])
        agu_sb = const_pool.tile([128, D // 128, 2 * R], dt.bfloat16)
        nc.sync.dma_start(agu_sb[:], a_gu.rearrange("(c p) r -> p c r", p=128))
        bg_sb = const_pool.tile([R, FS], dt.bfloat16)
        nc.sync.dma_start(bg_sb[:], b_g[:])
        bu_sb = const_pool.tile([R, FS], dt.bfloat16)
        nc.sync.dma_start(bu_sb[:], b_u[:])
        ad_sb = const_pool.tile([128, cfg.NFG, R], dt.bfloat16)
        nc.sync.dma_start(ad_sb[:], a_d.rearrange("(c p) r -> p c r", p=128))
        x3a_sb = const_pool.tile([R, T], dt.bfloat16)

        dq_pool = ctx.enter_context(tc.tile_pool(name="dq", bufs=2))
        # down-proj weights for q0/q1 (dequanted during phase-1 tail)
        wd0_pool = ctx.enter_context(tc.tile_pool(name="wd0",
                                                  bufs=2 * cfg.NFG))
        wd_ready = {}

        def chain_lut(X_f, S_f, w_f, fw):
            """LUT-dequant flat [128, fw] APs: u8 nibbles -> bf16 weights."""
            pool = dq_pool
            step = min(cfg.chain_fd, fw)
            for c0 in range(0, fw, step):
                cw = min(step, fw - c0)
                cs = slice(c0, c0 + cw)
                acc = pool.tile([128, step], dt.bfloat16, tag="dqa", name="acc")
                w = acc[:, 0:cw]
                nc.vector._custom_dve(
                    DEQ_PAIR, out=w, in0=X_f[:, cs],
                    s0=code_sb[:, 0:1], s1=code_sb[:, 1:2], imm2=0.0)
                for j in range(1, 8):
                    acc2 = pool.tile([128, step], dt.bfloat16, tag="dqa",
                                     name="acc2")
                    w2 = acc2[:, 0:cw]
                    nc.vector._custom_dve(
                        DEQ_PAIR_ACC, out=w2, in0=X_f[:, cs], in1=w,
                        s0=code_sb[:, 2 * j:2 * j + 1],
                        s1=code_sb[:, 2 * j + 1:2 * j + 2], imm2=float(2 * j))
                    w = w2
                nc.vector.tensor_tensor(w_f[:, cs], w, S_f[:, cs], ALU.mult)

        def dequant_block(wpool, idx_ap3, s_ap3, c2, fw, tag, nm,
                          wshape=None, wtag=None, xs_bufs=3):
            """Load c2 128-row chunks x fw cols, dequant as one wide plane.
            Returns the [128, c2, fw] weight tile."""
            pw = wshape and [128, wshape[0] * wshape[1]]
            X = dq_pool.tile([128, c2 * fw], dt.uint8, tag=f"X{tag}",
                             name=f"X{nm}", padded_shape=pw, bufs=xs_bufs)
            S = dq_pool.tile([128, c2 * fw], dt.bfloat16, tag=f"S{tag}",
                             name=f"S{nm}", padded_shape=pw, bufs=xs_bufs)
            dma_eng = nc.gpsimd if tag == "d" else None
            (dma_eng or nc.sync).dma_start(
                X.rearrange("p (c f) -> p c f", c=c2), idx_ap3)
            (dma_eng or nc.scalar).dma_start(
                S.rearrange("p (c f) -> p c f", c=c2), s_ap3)
            wt = wpool.tile([128, c2 * fw], dt.bfloat16,
                            tag=f"w{wtag or tag}", name=f"w{nm}",
                            padded_shape=pw)
            chain_lut(X[:], S[:], wt[:], c2 * fw)
            return wt.rearrange("p (c f) -> p c f", c=c2)

        def dq_phase2_gen(q, pool):
            """Dequant the down-proj chunks for quarter q (one plane/yield)."""
            dd0 = cfg.DDQ * q
            wd = {}
            for g in range(cfg.NFG):
                rows = slice(128 * g, 128 * (g + 1))
                cols = slice(dd0, dd0 + cfg.DDQ)
                wt = dequant_block(
                    pool,
                    d_idx[rows, cols].rearrange("(c p) f -> p c f", p=128),
                    d_s[rows, cols].rearrange("(c p) f -> p c f", p=128),
                    1, cfg.DDQ, "d", f"wd{q}_{g}", xs_bufs=6)
                wd[g] = wt[:, 0, :]
                yield 1
            wd_ready[q] = wd

        # =============== phase 1: gate/up matmuls -> x3 ===============
        with (
            tc.tile_pool(name="w", bufs=2 * (cfg.DP // 4) + 1) as w_pool,
            tc.tile_pool(name="xt", bufs=2) as xt_pool,
            tc.tile_pool(name="p1", bufs=2) as p1_pool,
            tc.tile_pool(name="ps1", bufs=2, space="PSUM") as psum1,
            tc.tile_pool(name="psa", bufs=2, space="PSUM") as psuma,
        ):
            # dequant generator: one gate-or-up 4-chunk block per next()
            C2B = 4
            def dq_slice_gen(f0, fw, wg, wu):
                for c0 in range(0, cfg.DP, C2B):
                    rows = slice(128 * c0, 128 * (c0 + C2B))
                    cols = slice(f0, f0 + fw)
                    for (ti, tsrc, wdict, nm) in (
                        (g_idx, g_s, wg, "g"), (u_idx, u_s, wu, "u"),
                    ):
                        wt = dequant_block(
                            w_pool,
                            ti[rows, cols].rearrange("(c p) f -> p c f",
                                                     p=128),
                            tsrc[rows, cols].rearrange("(c p) f -> p c f",
                                                       p=128),
                            C2B, fw, "gu", f"w{nm}{c0}",
                            wshape=[C2B, 384], wtag=nm)
                        for j in range(C2B):
                            wdict[c0 + j] = wt[:, j, :]
                        yield 1

            slices = cfg.f_slices
            wbufs = [({}, {}) for _ in slices]
            # slice 0 dequant up-front (overlaps the lora prepass MMs)
            for _ in dq_slice_gen(slices[0][0], slices[0][1],
                                  wbufs[0][0], wbufs[0][1]):
                pass

            # interleavable dequant work: slices 1..3, then down-proj q0
            def pending_dq():
                for s in range(1, len(slices)):
                    yield from dq_slice_gen(slices[s][0], slices[s][1],
                                            wbufs[s][0], wbufs[s][1])
                yield from dq_phase2_gen(0, wd0_pool)
                yield from dq_phase2_gen(1, wd0_pool)

            dq_iter = pending_dq()
            upsl = 2 * cfg.DP // C2B          # units per slice (16)
            dq_units = (len(slices) - 1) * upsl + 2 * cfg.NFG
            mm_groups_total = sum(fw // 128 for _, fw in slices) * cfg.NT
            done = [0, 0]  # units, groups

            def advance_dq(want):
                want = min(dq_units, want)
                while done[0] < want:
                    if next(dq_iter, None) is None:
                        done[0] = dq_units
                        break
                    done[0] += 1

            for s, (f0, fw) in enumerate(slices):
                # slice s's weights must be fully emitted before its MMs
                advance_dq(s * upsl)
                wg, wu = wbufs[s]
                ng = fw // 128
                for t in range(cfg.NT):
                    tt = slice(TT * t, TT * (t + 1))
                    xts_all = xt_pool.tile([128, cfg.DP, TT], dt.bfloat16,
                                           tag="xt", name="xts_all")
                    nc.sync.dma_start(
                        xts_all[:],
                        xT[:, tt].rearrange("(c p) t -> p c t", p=128))
                    xts = [xts_all[:, ci, :] for ci in range(cfg.DP)]
                    if s == 0:
                        # lora x@A for this token tile (overlaps dequant)
                        for ri, dst in ((0, xag_dram), (1, xau_dram)):
                            pa = psuma.tile([R, TT], dt.float32, tag="pa")
                            for ci in range(cfg.DP):
                                nc.tensor.matmul(
                                    pa[:], agu_sb[:, ci, R * ri:R * (ri + 1)],
                                    xts[ci][:],
                                    start=(ci == 0), stop=(ci == cfg.DP - 1))
                            st = p1_pool.tile([R, TT], dt.bfloat16, tag="st")
                            nc.scalar.copy(st[:], pa[:])
                            nc.scalar.dma_start(dst[:, tt], st[:])
                    xag_t = p1_pool.tile([R, TT], dt.bfloat16, tag="xag_t")
                    nc.sync.dma_start(xag_t[:], xag_dram[:, tt])
                    xau_t = p1_pool.tile([R, TT], dt.bfloat16, tag="xau_t")
                    nc.sync.dma_start(xau_t[:], xau_dram[:, tt])
                    pa2 = psuma.tile([R, TT], dt.float32, tag="pa2")
                    x3t_t = p1_pool.tile([128, ng, TT], dt.bfloat16,
                                         tag="x3t", padded_shape=[128, 3, TT])
                    for g in range(ng):
                        fg = slice(128 * g, 128 * (g + 1))
                        fga = slice(f0 + 128 * g, f0 + 128 * (g + 1))
                        pg = psum1.tile([128, TT], dt.float32, tag="pg")
                        pu = psum1.tile([128, TT], dt.float32, tag="pu")
                        for ci in range(cfg.DP):
                            nc.tensor.matmul(pg[:], wg[ci][:, fg], xts[ci][:],
                                             start=(ci == 0), stop=False)
                        nc.tensor.matmul(pg[:], bg_sb[:, fga], xag_t[:],
                                         start=False, stop=True)
                        for ci in range(cfg.DP):
                            nc.tensor.matmul(pu[:], wu[ci][:, fg], xts[ci][:],
                                             start=(ci == 0), stop=False)
                        nc.tensor.matmul(pu[:], bu_sb[:, fga], xau_t[:],
                                         start=False, stop=True)
                        sg = p1_pool.tile([128, TT], dt.bfloat16, tag="sg")
                        nc.scalar.activation(sg[:], pg[:], AFT.Silu)
                        pu_sb = p1_pool.tile([128, TT], dt.bfloat16,
                                             tag="pu_sb")
                        nc.scalar.copy(pu_sb[:], pu[:])
                        nc.vector.tensor_tensor(x3t_t[:, g, :], sg[:],
                                                pu_sb[:], ALU.mult)
                        done[1] += 1
                        advance_dq((done[1] * 3) // 4)
                    nc.scalar.dma_start(
                        x3_dram[f0:f0 + fw, tt].rearrange(
                            "(g p) t -> p g t", p=128),
                        x3t_t[:])
                    # x3a partial: accumulate a_d over this slice's groups
                    for g in range(ng):
                        g_abs = (f0 // 128) + g
                        nc.tensor.matmul(pa2[:], ad_sb[:, g_abs, :],
                                         x3t_t[:, g, :],
                                         start=(g == 0), stop=(g == ng - 1))
                    # fold the x3a partial into the running accumulator
                    x3a_p = p1_pool.tile([R, TT], dt.bfloat16, tag="x3ap")
                    nc.scalar.copy(x3a_p[:], pa2[:])
                    if s == 0:
                        nc.scalar.copy(x3a_sb[:, tt], x3a_p[:])
                    else:
                        nc.vector.tensor_tensor(x3a_sb[:, tt], x3a_sb[:, tt],
                                                x3a_p[:], ALU.add)
            for _ in dq_iter:   # drain remaining dequant (down-proj q0)
                pass

        # =============== phase 2: down matmul + ReduceScatter ===============
        with (
            tc.tile_pool(name="wd", bufs=2 * cfg.NFG + 2) as wd_pool,
            tc.tile_pool(name="p2", bufs=3) as p2_pool,
            tc.tile_pool(name="yb2", bufs=4) as yb_pool,
            tc.tile_pool(name="lor", bufs=2) as lor_pool,
            tc.tile_pool(name="ps2", bufs=6, space="PSUM") as psum2,
        ):
            ph2_iter = None
            ph2_done = [0]
            pending_rs = [None]

            def emit_rs():
                if pending_rs[0] is not None:
                    qq = pending_rs[0]
                    nc.gpsimd.collective_compute(
                        "ReduceScatter", ALU.add, replica_groups=rg,
                        ins=[rs_in[qq][:, :].opt()],
                        outs=[rs_out[qq][:, :].opt()],
                    )
                    pending_rs[0] = None

            for q in range(cfg.n_q):
                dd0 = cfg.DDQ * q
                wd = wd_ready[q]
                if q + 2 < cfg.n_q:
                    ph2_iter = dq_phase2_gen(q + 2, wd_pool)
                    ph2_done[0] = 0
                bdt = lor_pool.tile([R, cfg.DDQ], dt.bfloat16, tag="bdt")
                nc.sync.dma_start(bdt[:], b_d[:, dd0:dd0 + cfg.DDQ])
                n_tb = cfg.T // 512
                for tb in range(n_tb):
                    tsl = slice(512 * tb, 512 * (tb + 1))
                    x3b_all = p2_pool.tile([128, cfg.NFG, 512], dt.bfloat16,
                                           tag="x3l", name="x3b_all")
                    nc.sync.dma_start(
                        x3b_all[:],
                        x3_dram[:, tsl].rearrange("(g p) t -> p g t", p=128))
                    x3b = [x3b_all[:, g, :] for g in range(cfg.NFG)]
                    if tb == 1:
                        emit_rs()   # previous quarter's RS, after q's first
                                    # loads are already in the queues
                    for tk in range(4):
                        tg = 4 * tb + tk
                        tks = slice(128 * tk, 128 * (tk + 1))
                        tgs = slice(128 * tg, 128 * (tg + 1))
                        pd = psum2.tile([128, 512], dt.float32, tag="pd")
                        for g in range(cfg.NFG):
                            nc.tensor.matmul(
                                pd[:], x3b[g][:, tks], wd[g][:],
                                start=(g == 0), stop=False)
                        nc.tensor.matmul(
                            pd[:], x3a_sb[:, tgs], bdt[:],
                            start=False, stop=True)
                        yb = yb_pool.tile([128, cfg.DDQ], dt.bfloat16,
                                          tag="yb")
                        nc.scalar.copy(yb[:], pd[:])
                        nc.scalar.dma_start(rs_in[q][tgs, :], yb[:])
                    # pace the next-next slice's dequant
                    if ph2_iter is not None:
                        want = min(cfg.NFG, ((tb + 1) * cfg.NFG) // 5)
                        while ph2_done[0] < want:
                            if next(ph2_iter, None) is None:
                                ph2_done[0] = cfg.NFG
                                break
                            ph2_done[0] += 1
                if ph2_iter is not None:
                    for _ in ph2_iter:
                        pass
                    ph2_iter = None
                pending_rs[0] = q
            emit_rs()
            for q in range(cfg.n_q):
                nc.sync.dma_start(y_q[q][:, :], rs_out[q][:, :])

    nc.compile()
    return nc


# ----------------- host side -----------------

_CACHE = {}


def _get_graph(cfg: Cfg):
    key = (cfg.D, cfg.T, cfg.F, cfg.ncores)
    if key not in _CACHE:
        _CACHE[key] = build_graph(cfg)
    return _CACHE[key]


def _unpack_idx(packed, rows, cols):
    """bnb layout: byte i of row-major [rows, cols] holds elements (2i, 2i+1)
    as (hi, lo) nibbles. Returns [rows, cols] u8 of 4-bit indices."""
    b = (packed.astype(np.int64) & 0xFF).astype(np.uint8).reshape(rows, cols // 2)
    out = np.empty((rows, cols), np.uint8)
    out[:, 0::2] = b >> 4
    out[:, 1::2] = b & 0xF
    return out


def _prep_inputs(cfg: Cfg, inputs):
    """Shard + lay out the full inputs for each core. Marshalling only."""
    D, T, F, FP, FS, R = cfg.D, cfg.T, cfg.F, cfg.FP, cfg.FS, cfg.R
    blk = cfg.block

    x = inputs["x"]
    xT = np.ascontiguousarray(np.asarray(x).T).astype(BF16)

    gi_full = _unpack_idx(np.asarray(inputs["w_gate_packed"]), F, D)
    ui_full = _unpack_idx(np.asarray(inputs["w_up_packed"]), F, D)
    di_full = _unpack_idx(np.asarray(inputs["w_down_packed"]), D, F)
    ga_full = np.asarray(inputs["w_gate_absmax"]).reshape(F, D // blk).astype(np.float32)
    ua_full = np.asarray(inputs["w_up_absmax"]).reshape(F, D // blk).astype(np.float32)
    da_full = np.asarray(inputs["w_down_absmax"]).reshape(D, F // blk).astype(np.float32)

    code_rep = np.broadcast_to(
        np.asarray(inputs["code"]).astype(np.float32)[None, :], (128, 16)
    ).copy()
    a_gu = np.concatenate(
        [inputs["w_gate_lora_a"], inputs["w_up_lora_a"]], axis=1
    ).astype(BF16)

    def pad_cols(m, w):
        return np.concatenate(
            [m, np.zeros((m.shape[0], w - m.shape[1]), m.dtype)], 1)

    b_g_full = pad_cols(np.asarray(inputs["w_gate_lora_b"], np.float32), FP)
    b_u_full = pad_cols(np.asarray(inputs["w_up_lora_b"], np.float32), FP)
    a_d_full = np.concatenate(
        [np.asarray(inputs["w_down_lora_a"], np.float32),
         np.zeros((FP - F, R), np.float32)], 0)
    b_d = np.asarray(inputs["w_down_lora_b"]).astype(BF16)

    in_maps = []
    for i in range(cfg.ncores):
        f0, f1 = FS * i, FS * (i + 1)
        nf = max(0, min(f1, F) - f0)        # valid f rows in this shard

        def row_shard(idx_full, am_full):
            """gate/up: -> idx [D, FS] u8, S [D, FS] bf16."""
            gi = np.zeros((FS, D), np.uint8)
            gs = np.zeros((FS, D), np.float32)
            if nf > 0:
                gi[:nf] = idx_full[f0:f0 + nf]
                gs[:nf] = np.repeat(am_full[f0:f0 + nf], blk, axis=1)
            return (np.ascontiguousarray(gi.T),
                    np.ascontiguousarray(gs.T.astype(BF16)))

        def col_shard(idx_full, am_full):
            """down: -> idx [FS, D] u8, S [FS, D] bf16."""
            di = np.zeros((D, FS), np.uint8)
            dsc = np.zeros((D, FS), np.float32)
            if nf > 0:
                di[:, :nf] = idx_full[:, f0:f0 + nf]
                nb = (nf + blk - 1) // blk
                amr = np.repeat(am_full[:, f0 // blk:f0 // blk + nb],
                                blk, axis=1)[:, :nf]
                dsc[:, :nf] = amr
            return (np.ascontiguousarray(di.T),
                    np.ascontiguousarray(dsc.T.astype(BF16)))

        g_i, g_sc = row_shard(gi_full, ga_full)
        u_i, u_sc = row_shard(ui_full, ua_full)
        d_i, d_sc = col_shard(di_full, da_full)

        in_maps.append({
            "xT": xT,
            "g_idx": g_i, "g_s": g_sc,
            "u_idx": u_i, "u_s": u_sc,
            "d_idx": d_i, "d_s": d_sc,
            "code_rep": code_rep,
            "a_gu": a_gu,
            "b_g": np.ascontiguousarray(b_g_full[:, f0:f1]).astype(BF16),
            "b_u": np.ascontiguousarray(b_u_full[:, f0:f1]).astype(BF16),
            "a_d": np.ascontiguousarray(a_d_full[f0:f1]).astype(BF16),
            "b_d": b_d,
        })
    return in_maps


def run(cfg: Cfg, inputs, trace=False, **kwargs):
    nc = _get_graph(cfg)
    in_maps = _prep_inputs(cfg, inputs)
    res = run_bass_kernel_spmd(
        nc, in_maps, core_ids=list(range(cfg.ncores)), trace=trace, **kwargs
    )
    y = np.concatenate(
        [np.concatenate([res.results[i][f"y_q{j}"] for j in range(cfg.n_q)],
                        axis=1)
         for i in range(cfg.ncores)], 0)
    return y, res


def kernel(**inputs) -> np.ndarray:
    cfg = Cfg()
    y, _ = run(cfg, inputs)
    return y.astype(np.float32)
